# revision 62
# baseline (speedup 1.0000x reference)
"""Grouped GEMM (MoE expert matmul) on 8 TRN2 NeuronCores.

Problem: a [66048, 1024] f32 tokens, b [8, 1024, 1024] f32 expert weights,
static uneven per-expert token counts. d[m] = a[m] @ b[expert(m)].

Strategy (expert-parallel via M-sharding, zero collectives):
- Token rows are assigned host-side to 8 cores x 3 "slots" of (6, 22, 37)
  m-tiles (128 rows each) = 65 tiles/core. Every slot is single-expert;
  each core receives the 3 expert matrices its slots need. The
  (core,slot)->expert binding is pure DATA, so one SPMD program serves
  all cores. Only 4 of 520 tiles are zero-padding.
- A is pre-transposed host-side into per-tile lhsT layout [ki, ko, mm]
  (so the PE does no transposes at all) and split into fp8-e4m3 hi+lo
  (a ~= a_h + a_l); B likewise. The product is computed as
      d ~= a_h@b_h + a_l@b_h + a_h@b_l
  with all terms as fp8 DoubleRow matmuls (2 k-tiles per instruction,
  0.5 cycles/row) accumulating into the same PSUM bank. The b_l
  correction covers 2 of 4 k-pairs, dropped to 1 in 12 late tiles
  (BLP): rel err 1.968e-2 measured vs the 2e-2 gate, at 18-20 matmul
  instructions per tile (bf16-equivalent cost 32) -> per-core PE floor
  ~136.4us (1276 matmuls x 106.93ns).
- Per m-tile: 18-20 DoubleRow matmuls (2 psum halves x (4+4+blp)
  k-pair chains), PSUM evicted to SBUF as bf16 by DVE, stored by HWDGE
  DMA on the sync queue; d upcast host-side. All loads run on HWDGE
  queues (sync/scalar alternating): HWDGE descriptor-gen cadence is
  ~625ns/piece vs the SWDGE prep pipeline's ~1.04us, so ~728ns startup
  pieces flow back-to-back at full DMA bandwidth with each piece's
  +900ns completion semaphore firing as early as possible. Tiles 0-3
  are emitted jj-wavefront-major in exactly the startup pieces' arrival
  order (b0h ko0-1, a-hi t0t1, b0h ko2-3, a-hi t2t3, b0h ko4-5/6-7,
  a-lo halves, b0l) — the Tile scheduler is a priority list scheduler
  with priority = emission order, and the PE executes in the scheduled
  order, so emission order must match data arrival for a dense stream.
  29+1 warmup matmuls on zeroed tiles burn the PE p-state ramp (full
  clock needs ~3us of continuous execution) and end at ~4.95us, the
  anchor-derived stream start (a-hi t2/t3 semaphore minus 8 matmuls). Pieces whose relative wire
  order matters (b1l vs the chunk-1 lo pieces) share one queue: the two
  HWDGE queues' SEQs run independently, so cross-queue request order
  can invert. The last tile's nh1 runs as a 384- and a 128-wide chain
  with evictions split across DVE/Activation and one merged [512:]
  store (HWDGE gens serialize on one shared device, so fewer tail
  stores = shorter tail).
  Measured (TimelineSim): 145392 ns = 5.0us startup + PE stream at the
  106.7ns/matmul DR floor with one 38ns stall (the a-hi t2/t3
  semaphore, the stream's anchor) + 3.9us tail.
"""

import numpy as np

GROUP_SIZES = [12288, 10240, 9216, 8192, 7168, 7168, 6144, 5632]
OFFSETS = np.concatenate([[0], np.cumsum(GROUP_SIZES)]).astype(np.int64)
M_TOTAL = int(OFFSETS[-1])  # 66048
K = 1024
N = 1024
E = 8
P = 128
KK = K // P  # 8 k-tiles
NH = 2  # two 512-wide psum halves

# Per-core uniform slot structure, in m-tiles of 128 rows.
SLOT_TILES = (6, 22, 37)  # sum = 65 tiles = 8320 rows per core
TILES_PER_CORE = sum(SLOT_TILES)
ROWS_PER_CORE = TILES_PER_CORE * P
SLOT_ROW_OFF = (0, SLOT_TILES[0] * P, (SLOT_TILES[0] + SLOT_TILES[1]) * P)

CHUNK = 5  # m-tiles per A-load DMA; 13 chunks cover 65 tiles
NCHUNKS = TILES_PER_CORE // CHUNK
PREFETCH = 3  # chunks issued ahead of consumption
# Dummy PE matmuls burning the PE p-state ramp during the startup DMA:
# 29 full-width instrs end at ~4.91us (the full-clock point); one final
# narrow (104-wide, ~43ns) warmup pads to ~4.95us so the first 8 real
# matmuls run dense exactly into the a-hi t2/t3 semaphore at ~5.81us
# with every real instruction billing at full clock.
WARMUP = 29

# Per-tile a_h@b_l coverage in k-pairs (of 4). blp=2 everywhere plus
# blp=1 on 12 late tiles: emulated rel err 1.961e-2, measured on HW
# 1.968e-2 vs the 2e-2 gate (the emulation->HW factor is a consistent
# x1.004), at 1276 DR matmuls/core (was 1350 for the (3,2) mix).
BLP = tuple(
    1 if (t >= 42 and t % 2 == 0) else 2 for t in range(TILES_PER_CORE)
)
# b_l k-tiles actually loaded: max(blp) pairs = ko 0..3.
BL_KO = 2 * max(BLP)

# expert id for (slot, core): found by exact-cover search; 4 pad tiles total.
SLOT_EXPERT = (
    (1, 3, 4, 4, 5, 5, 6, 6),  # slot 0: 6 tiles each
    (0, 3, 4, 4, 5, 5, 7, 7),  # slot 1: 22 tiles each
    (0, 0, 1, 1, 2, 2, 3, 6),  # slot 2: 37 tiles each
)


def _build_schedule():
    """Returns list of (core, slot, slot_row_start, global_row_start, nrows)."""
    cursor = [int(OFFSETS[e]) for e in range(E)]
    recs = []
    # Deterministic fill order: slot index, then core.
    for s in range(3):
        for c in range(8):
            e = SLOT_EXPERT[s][c]
            cap = SLOT_TILES[s] * P
            take = min(cap, int(OFFSETS[e + 1]) - cursor[e])
            if take > 0:
                recs.append((c, s, SLOT_ROW_OFF[s], cursor[e], take))
                cursor[e] += take
    for e in range(E):
        assert cursor[e] == int(OFFSETS[e + 1]), (e, cursor[e])
    return recs


_SCHEDULE = _build_schedule()


def _build_bass():
    import concourse.bass as bass  # noqa: F401
    import concourse.mybir as mybir
    import concourse.tile as tile
    from concourse import bacc

    f32 = mybir.dt.float32
    bf16 = mybir.dt.bfloat16
    f8 = mybir.dt.float8e4

    nc = bacc.Bacc(
        "TRN2", target_bir_lowering=False, debug=False, enable_asserts=False
    )

    # A in pre-transposed lhsT layout: row (t*128 + ki) holds the 1024
    # values [ko, mm] of tile t; hi and lo fp8 planes.
    ah = nc.dram_tensor("ah", [ROWS_PER_CORE, K], f8, kind="ExternalInput").ap()
    al = nc.dram_tensor("al", [ROWS_PER_CORE, K], f8, kind="ExternalInput").ap()
    # B per slot: row (ki*8 + ko) holds the 1024 n-values; hi and lo.
    bhs = [
        nc.dram_tensor(f"bh{s}", [P * KK, N], f8, kind="ExternalInput").ap()
        for s in range(3)
    ]
    bls = [
        nc.dram_tensor(f"bl{s}", [P * KK, N], f8, kind="ExternalInput").ap()
        for s in range(3)
    ]
    d = nc.dram_tensor("d", [ROWS_PER_CORE, N], bf16, kind="ExternalOutput").ap()

    # which slot (-> b input) each m-tile uses (static, uniform across cores)
    tile_slot = []
    for s in range(3):
        tile_slot += [s] * SLOT_TILES[s]

    from contextlib import ExitStack

    with tile.TileContext(nc) as tc, ExitStack() as ctx:
        bpool = ctx.enter_context(tc.tile_pool(name="bpool", bufs=1))
        apool = ctx.enter_context(tc.tile_pool(name="apool", bufs=4))
        psd = ctx.enter_context(tc.tile_pool(name="psd", bufs=8, space="PSUM"))
        # Deep store staging: early DMA-engine time is monopolized by the
        # B/A loads, so d-stores queue up; 24 bufs (48KB) of slack keep the
        # eviction copies (and thus PSUM recycling) from backpressuring PE.
        dpool = ctx.enter_context(tc.tile_pool(name="dpool", bufs=24))

        # First load issued before anything else: b0h[ko0-1] on the sync
        # HWDGE queue, so SP's DMA issue chain starts at t=0.
        HB = KK * N // 2
        HQ = HB // 2
        bt00 = bpool.tile([P, KK, N], f8, name="b0_0")
        b00f = bt00[:].rearrange("ki ko n -> ki (ko n)")
        b00in = bhs[0].rearrange("(ki ko) n -> ki (ko n)", ko=KK)
        nc.sync.dma_start(out=b00f[:, :HQ], in_=b00in[:, :HQ])

        # Warmup: the PE p-state ramps to full clock only after 3us of
        # continuous execution. Dummy DoubleRow matmuls on zeroed tiles
        # keep the PE busy (and ramping) while the first B/A DMAs land,
        # so the real matmul stream starts at full speed with no idle gap.
        wa = bpool.tile([P, 2, P], f8, name="wa")
        wb = bpool.tile([P, 2, 256], f8, name="wb")
        nc.vector.memset(wa[:], 0.0)
        nc.vector.memset(wb[:], 0.0)
        wp = psd.tile([P, 512], f32, name="ps")
        for _ in range(WARMUP):
            nc.tensor.matmul(
                wp[:, :256],
                wa[:],
                wb[:],
                start=True,
                stop=True,
                perf_mode=mybir.MatmulPerfMode.DoubleRow,
            )
        nc.tensor.matmul(
            wp[:, :104],
            wa[:],
            wb[:, :, :104],
            start=True,
            stop=True,
            perf_mode=mybir.MatmulPerfMode.DoubleRow,
        )

        b_sb = {}  # (slot, lvl) -> [128, KK, N] fp8 tile

        def load_b(s, lvl, pieces=None, queues=None):
            src = (bhs if lvl == 0 else bls)[s]
            bt = b_sb.get((s, lvl))
            if bt is None:
                bt = bpool.tile([P, KK, N], f8, name=f"b{lvl}_{s}")
            # Flat [128, 8KB] view: per-partition lines are contiguous in
            # both DRAM and SBUF, so the DMA needs 128 descriptors, not
            # 1024 — shorter SWDGE descriptor-generation on the Pool SEQ.
            # Lo planes: ko-tiles >= BL_KO are never read (the b_l
            # correction covers at most BL_KO/2 k-pairs), so don't load
            # them.
            out_f = bt[:].rearrange("ki ko n -> ki (ko n)")
            in_f = src.rearrange("(ki ko) n -> ki (ko n)", ko=KK)
            end = (KK if lvl == 0 else BL_KO) * N
            pieces = pieces or [(0, end)]
            for i, (p0, p1) in enumerate(pieces):
                queue = (queues or {}).get(i, nc.sync)
                queue.dma_start(out=out_f[:, p0:p1], in_=in_f[:, p0:p1])
            b_sb[(s, lvl)] = bt

        a_ch = {}  # (chunk, lvl) -> [128, CHUNK, KK, 128] fp8 tile

        def load_chunk(c, pieces=None, queues=None, lvls=(0, 1)):
            # pieces: tile sub-ranges loaded as separate DMAs so their
            # consumers (tracked per sub-tile range) unblock early.
            for lvl in lvls:
                pool, src = ((apool, ah), (apool, al))[lvl]
                at = a_ch.get((c, lvl))
                if at is None:
                    at = pool.tile([P, CHUNK, KK, P], f8, name=f"a{lvl}")
                    a_ch[(c, lvl)] = at
                for i, (p0, p1) in enumerate(pieces or [(0, CHUNK)]):
                    queue = (queues or {}).get(i, nc.sync)
                    queue.dma_start(
                        out=at[:, p0:p1],
                        in_=src[
                            (c * CHUNK + p0) * P : (c * CHUNK + p1) * P, :
                        ].rearrange("(c ki) (ko mm) -> ki c ko mm", ki=P, ko=KK),
                    )

        # Startup: the serialized DMA train is ordered so that each piece
        # lands just before its first consumer instructions need it, with
        # the two lead pieces on HWDGE queues (shorter issue lead than a
        # SWDGE prep). Desired grant order: b0h[ko0-3] (sync), a0_hi
        # tiles0-1 (scalar), then the SWDGE train: b0h[ko4-7], a0_lo
        # tiles0-1, b0l in halves, a0 tiles2-4, b1, chunks 1-2, b2 later.
        # Grant order on the serialized DMA engine follows request order:
        # b0h[ko0-3] via sync HWDGE (shortest lead), then the SWDGE preps
        # in emission order. The first A piece is prepped BEFORE b0h's
        # second half so the first matmuls (jj0/jj1 of tiles 0-1) can
        # start ~1.4us earlier; the scheduler hoists them over the wait
        # for b0h[ko4-7].
        b_sb[(0, 0)] = bt00
        # Startup pieces are split finer than bandwidth needs: each DMA's
        # completion semaphore fires +900ns after ITS transfer ends, so a
        # split's first half unblocks consumers ~730ns earlier while the
        # transfers occupy the same serialized DMA window (pure latency
        # win, no downstream shift). With the splits below the PE rolls
        # from the warmup matmuls straight into the real stream with no
        # data stalls.
        # The whole startup train runs on HWDGE queues (sync/scalar
        # alternating): HWDGE descriptor-gen cadence is 625ns/piece vs
        # the SWDGE prep pipeline's ~1.04us/piece on Pool.ENGINE, and
        # gens pace the shared-DMA request order, so ~728ns pieces flow
        # back-to-back at full bandwidth with each piece's +900ns
        # completion semaphore firing as early as possible. Pieces are
        # ordered by first-consumer time; the wavefront emission below
        # consumes them in exactly this order.
        load_chunk(0, pieces=[(0, 2)], lvls=(0,), queues={0: nc.scalar})
        nc.sync.dma_start(out=b00f[:, HQ:HB], in_=b00in[:, HQ:HB])
        load_chunk(0, pieces=[(2, 4)], lvls=(0,), queues={0: nc.scalar})
        nc.sync.dma_start(out=b00f[:, HB : HB + HQ], in_=b00in[:, HB : HB + HQ])
        nc.scalar.dma_start(out=b00f[:, HB + HQ :], in_=b00in[:, HB + HQ :])
        load_chunk(0, pieces=[(0, 2)], lvls=(1,), queues={0: nc.sync})
        load_chunk(0, pieces=[(2, 4)], lvls=(1,), queues={0: nc.scalar})
        load_b(0, 1, queues={0: nc.sync})
        load_chunk(0, pieces=[(4, CHUNK)], lvls=(0,), queues={0: nc.scalar})
        load_chunk(0, pieces=[(4, CHUNK)], lvls=(1,), queues={0: nc.sync})
        load_chunk(1, pieces=[(0, 1)], lvls=(0,), queues={0: nc.scalar})
        load_chunk(1, pieces=[(0, 1)], lvls=(1,), queues={0: nc.sync})
        load_chunk(1, pieces=[(1, CHUNK)], lvls=(0,), queues={0: nc.scalar})
        load_b(1, 0, pieces=[(0, HB), (HB, KK * N)], queues={0: nc.sync, 1: nc.scalar})
        load_b(1, 1, pieces=[(0, BL_KO * N // 2)], queues={0: nc.sync})
        load_b(
            1,
            1,
            pieces=[(BL_KO * N // 2, BL_KO * N)],
            queues={0: nc.sync},
        )
        load_chunk(1, pieces=[(1, 2)], lvls=(1,), queues={0: nc.sync})
        load_chunk(1, pieces=[(2, CHUNK)], lvls=(1,), queues={0: nc.sync})
        load_chunk(2, queues={0: nc.sync})

        # Tiles 0-3 are emitted jj-wavefront-major (all tiles' jj0, then
        # all jj1, ...): the Tile scheduler is a priority list scheduler
        # with priority = emission order, so this makes its chosen PE
        # order match the startup DMA arrival order (a-hi pieces ->
        # b0h ko4-7 -> a-lo -> b0l) and the stream runs dense from the
        # first real matmul instead of stalling mid-tile.
        WAVE_TILES = 4
        load_chunk(PREFETCH)  # the t=0 chunk prefetch, same position
        wave_ps = [
            (
                psd.tile([P, 512], f32, name="ps"),
                psd.tile([P, 512], f32, name="ps"),
            )
            for _ in range(WAVE_TILES)
        ]
        wat_h = a_ch[(0, 0)]
        wat_l = a_ch[(0, 1)]
        wb_h = b_sb[(0, 0)]
        wb_l = b_sb[(0, 1)]
        wblp = BLP[0]
        assert all(BLP[t] == wblp for t in range(WAVE_TILES))
        assert all(tile_slot[t] == 0 for t in range(WAVE_TILES))
        # Wave order matches startup-piece arrival: a-hi t0t1, a-hi t2t3,
        # b0h ko4-5, ko6-7, a-lo halves, b0l.
        waves = (
            [("h", jj, (0, 2)) for jj in (0, 1)]
            + [("h", jj, (2, 4)) for jj in (0, 1)]
            + [("h", jj, (0, 4)) for jj in (2, 3)]
            + [("l", jj, (0, 2)) for jj in range(KK // 2)]
            + [("l", jj, (2, 4)) for jj in range(KK // 2)]
            # t0 finishes both b_l pairs first so its PSUM banks evict
            # early: tile 4's chains reuse them and start without waiting.
            + [("b", jj, (0, 1)) for jj in range(wblp)]
            + [("b", jj, (1, 4)) for jj in range(wblp)]
        )
        for wi, (term, jj, (t0, t1)) in enumerate(waves):
            for t in range(t0, t1):
                w_t = wat_l if term == "l" else wat_h
                r_t = wb_l if term == "b" else wb_h
                for n0, n1 in ((0, 512), (512, 1024)):
                    nc.tensor.matmul(
                        wave_ps[t][n0 // 512][:],
                        w_t[:, t, 2 * jj : 2 * jj + 2, :],
                        r_t[:, 2 * jj : 2 * jj + 2, n0:n1],
                        start=(term == "h" and jj == 0),
                        stop=(term == "b" and jj == wblp - 1),
                        perf_mode=mybir.MatmulPerfMode.DoubleRow,
                    )
        for t in range(WAVE_TILES):
            d_sb = dpool.tile([P, N], bf16, name="d_sb")
            nc.vector.tensor_copy(d_sb[:, :512], wave_ps[t][0][:])
            nc.vector.tensor_copy(d_sb[:, 512:], wave_ps[t][1][:])
            nc.sync.dma_start(out=d[t * P : (t + 1) * P, :], in_=d_sb[:])

        for t in range(WAVE_TILES, TILES_PER_CORE):
            c, j = divmod(t, CHUNK)
            if j == 0 and c + PREFETCH < NCHUNKS:
                load_chunk(c + PREFETCH)
            if t == 8:
                load_b(2, 0)
                load_b(2, 1)
            s = tile_slot[t]
            at_h = a_ch[(c, 0)]
            at_l = a_ch[(c, 1)]
            b_h = b_sb[(s, 0)]
            b_l = b_sb[(s, 1)]
            last = t == TILES_PER_CORE - 1
            ps0 = psd.tile([P, 512], f32, name="ps")
            if not last:
                ps1 = psd.tile([P, 512], f32, name="ps")
                chains = ((ps0[:], 0, 512), (ps1[:], 512, 1024))
            else:
                # Last tile: nh1 as a 384- plus a 128-wide chain (same PE
                # cycles) so each piece's eviction+store pipeline overlaps
                # the later chains' matmuls and the final piece in the
                # store critical path is small.
                psq0 = psd.tile([P, 512], f32, name="ps")
                psq1 = psd.tile([P, 512], f32, name="ps")
                chains = (
                    (ps0[:], 0, 512),
                    (psq0[:, :384], 512, 896),
                    (psq1[:, :128], 896, 1024),
                )
            blp = BLP[t]
            nchain = 2 * (KK // 2) + blp
            for pst, n0, n1 in chains:
                idx = 0
                for w_t, r_t, npairs in (
                    (at_h, b_h, KK // 2),
                    (at_l, b_h, KK // 2),
                    (at_h, b_l, blp),
                ):
                    for jj in range(npairs):
                        nc.tensor.matmul(
                            pst,
                            w_t[:, j, 2 * jj : 2 * jj + 2, :],
                            r_t[:, 2 * jj : 2 * jj + 2, n0:n1],
                            start=(idx == 0),
                            stop=(idx == nchain - 1),
                            perf_mode=mybir.MatmulPerfMode.DoubleRow,
                        )
                        idx += 1
            d_sb = dpool.tile([P, N], bf16, name="d_sb")
            if not last:
                nc.vector.tensor_copy(d_sb[:, :512], ps0[:])
                nc.vector.tensor_copy(d_sb[:, 512:], ps1[:])
                nc.sync.dma_start(out=d[t * P : (t + 1) * P, :], in_=d_sb[:])
            else:
                # Each piece is evicted as soon as its chain stops; only
                # the final 128-wide piece trails the last matmul. That
                # piece is evicted on the (idle) Activation engine in
                # parallel with the 384-wide DVE eviction, and both feed
                # ONE merged [512:] store: HWDGE descriptor-gens serialize
                # on a single shared device, so two tail stores would cost
                # an extra ~630ns of gen latency.
                # ps0's eviction goes on Activation: DVE still has the
                # previous tile's eviction queued (and a DVE.SEQ
                # instruction holds the SEQ during its wait, so adding
                # work to DVE here delays the psq0 eviction), and this
                # store's HWDGE gen gates the merged store's gen below.
                nc.scalar.copy(d_sb[:, :512], ps0[:])
                nc.sync.dma_start(
                    out=d[t * P : (t + 1) * P, :512], in_=d_sb[:, :512]
                )
                nc.vector.tensor_copy(d_sb[:, 512:896], psq0[:, :384])
                nc.scalar.copy(d_sb[:, 896:], psq1[:, :128])
                nc.sync.dma_start(
                    out=d[t * P : (t + 1) * P, 512:], in_=d_sb[:, 512:]
                )
            # free the chunk dict entries we no longer need
            if j == CHUNK - 1:
                a_ch.pop((c, 0), None)
                a_ch.pop((c, 1), None)

    nc.compile()
    return nc


_NC_CACHE = None


def _prep_inputs(a, b):
    """Host-side shard + transpose + fp8 hi/lo split. Returns in_maps."""
    import ml_dtypes

    f8 = ml_dtypes.float8_e4m3

    a32 = np.ascontiguousarray(np.asarray(a), dtype=np.float32)
    b32 = np.ascontiguousarray(np.asarray(b), dtype=np.float32)
    assert a32.shape == (M_TOTAL, K), a32.shape
    assert b32.shape == (E, K, N), b32.shape

    a_h = a32.astype(f8)
    a_l = (a32 - a_h.astype(np.float32)).astype(f8)
    b_h = b32.astype(f8)
    b_l = (b32 - b_h.astype(np.float32)).astype(f8)

    # Per-expert B in [ki, ko, n] lhs-contraction layout, flattened 2D.
    def prep_b(x):  # x: [K, N] fp8
        return np.ascontiguousarray(
            x.reshape(KK, P, N).transpose(1, 0, 2).reshape(P * KK, N)
        )

    b_h_prep = [prep_b(b_h[e]) for e in range(E)]
    b_l_prep = [prep_b(b_l[e]) for e in range(E)]

    # Per-core A shards (zero-padded), then per-tile transpose to
    # [t, ki, ko, mm] flattened to [(t ki), (ko mm)].
    def prep_a(x):  # x: [ROWS_PER_CORE, K] fp8
        y = x.reshape(TILES_PER_CORE, P, KK, P).transpose(0, 3, 2, 1)
        return np.ascontiguousarray(y).reshape(ROWS_PER_CORE, K)

    in_maps = []
    for c in range(8):
        sh_h = np.zeros((ROWS_PER_CORE, K), dtype=f8)
        sh_l = np.zeros((ROWS_PER_CORE, K), dtype=f8)
        for cc, s, soff, goff, n in _SCHEDULE:
            if cc == c:
                sh_h[soff : soff + n] = a_h[goff : goff + n]
                sh_l[soff : soff + n] = a_l[goff : goff + n]
        m = {"ah": prep_a(sh_h), "al": prep_a(sh_l)}
        for s in range(3):
            e = SLOT_EXPERT[s][c]
            m[f"bh{s}"] = b_h_prep[e]
            m[f"bl{s}"] = b_l_prep[e]
        in_maps.append(m)
    return in_maps


def kernel(a, b):
    global _NC_CACHE
    from concourse.bass_utils import run_bass_kernel_spmd

    if _NC_CACHE is None:
        _NC_CACHE = _build_bass()
    nc = _NC_CACHE

    in_maps = _prep_inputs(a, b)
    res = run_bass_kernel_spmd(nc, in_maps, core_ids=list(range(8)))

    out = np.empty((M_TOTAL, N), dtype=np.float32)
    for c, s, soff, goff, n in _SCHEDULE:
        out[goff : goff + n] = res.results[c]["d"][soff : soff + n].astype(
            np.float32
        )
    return out

